# revision 46
# baseline (speedup 1.0000x reference)
"""Trainium2 Bass kernel for causal multi-head attention with interleaved RoPE.

Problem: B=2, S=2048, D=1024, 16 heads x 64 dims, causal, rope theta=1e4.

Sharding (8 cores): 2-way batch x 4-way head tensor-parallel.
  core i: batch b = i // 4, head group g = i % 4 (heads 4g..4g+3, dims 256).
  Each core computes q/k/v for its heads from x[b], runs causal flash
  attention, and produces a partial output projection outT = wo_g.T-slice
  contribution [D, S].  Host sums the 4 partials per batch and transposes.

v2 layout notes:
  - All matmul operands are bf16 (PSUM accumulation stays f32); tolerance is
    2e-2 so bf16's ~0.2% error is far inside budget.  bf16 streams at 1
    cycle/row on the PE for any N and halves SBUF/DMA traffic.
  - qT/kT are computed transposed ([dim, row]) via lhsT=weights, rhs=x^T.
  - RoPE pairs are de-interleaved on the host by permuting weight columns so
    pair partners sit 16 partitions apart (within a 32-partition quadrant),
    making the partner fetch a single DVE stream_shuffle.
  - Scores are computed transposed (S^T[k, q]) so the AV matmul needs no
    transposes; the softmax normalizer comes from a ones-row appended to V.
  - Causality: only k-chunks up to the diagonal are processed; diagonal-band
    128x128 blocks get a -60 triangular mask add before exp.
  - Attention is software-pipelined: the AV matmul for chunk kc is emitted
    after the score matmuls for chunk kc+1, so the Tensor engine never waits
    on the Scalar-engine exp.
  - Normalization is per-(qt,g) and stays on-chip: sums row -> K=1 broadcast
    matmul -> reciprocal_approx_fast -> fused multiply into bf16 oT.
"""

import os
import sys

sys.path.insert(0, "/opt/trn_rl_repo")

import numpy as np
import ml_dtypes

B = 2
S = 2048
D = 1024
NH = 16
HD = 64
THETA = 10000.0
NCORES = 8
HPC = 4  # heads per core
DC = HPC * HD  # 256 dims per core
GQ = 2  # 128-partition groups per core for q/k/o dims (DC/128)
QT = 512  # query tile (free dim)
NQT = S // QT
KC = 128  # key chunk (partition dim)
NKC = S // KC
NDK = D // 128  # contraction chunks for projections
MASKVAL = -60.0
BF16 = ml_dtypes.bfloat16

_CACHE = {}


def _install_axon_ntff_hook():
    """Register antenv.axon_hooks so trace=True (BASS_TRACE=1) works."""
    import types

    if "antenv.axon_hooks" in sys.modules:
        return
    m = types.ModuleType("antenv.axon_hooks")
    _hook = [None]
    m.set_axon_ntff_profile_hook = lambda h: _hook.__setitem__(0, h)
    m.get_axon_ntff_profile_hook = lambda: _hook[0]
    sys.modules["antenv.axon_hooks"] = m
    try:
        import antenv

        antenv.axon_hooks = m
        from trn_agent_boot.trn_boot import _ntff_profile_via_ctypes

        hook = _ntff_profile_via_ctypes("/opt/axon/libaxon_pjrt.so")
        if hook is not None:
            m.set_axon_ntff_profile_hook(hook)
    except Exception:
        pass


def _rope_perm_local():
    """Permutation of one head's 64 dims: original interleaved pair (2i, 2i+1)
    -> t0 at quadrant*32 + (i%16), t1 at quadrant*32 + 16 + (i%16), with
    quadrant = i // 16.  Returns perm such that new[j] = old[perm[j]]."""
    perm = np.zeros(HD, dtype=np.int64)
    for i in range(HD // 2):
        qd, r = divmod(i, 16)
        perm[qd * 32 + r] = 2 * i
        perm[qd * 32 + 16 + r] = 2 * i + 1
    return perm


def _rope_tables():
    """cos_dup/sin_signed [128, S]: per-partition rope tables matching the
    de-interleaved layout (pattern repeats every 64 partitions)."""
    inv_freq = 1.0 / (THETA ** (np.arange(0, HD, 2, dtype=np.float64) / HD))  # [32]
    pos = np.arange(S, dtype=np.float64)
    ang = pos[None, :] * inv_freq[:, None]  # [32, S]
    cos = np.cos(ang)
    sin = np.sin(ang)
    cos_dup = np.zeros((128, S), dtype=np.float32)
    sin_signed = np.zeros((128, S), dtype=np.float32)
    for p in range(128):
        d = p % HD
        qd, r0 = divmod(d, 32)
        if r0 < 16:
            i = qd * 16 + r0
            cos_dup[p] = cos[i]
            sin_signed[p] = -sin[i]
        else:
            i = qd * 16 + (r0 - 16)
            cos_dup[p] = cos[i]
            sin_signed[p] = sin[i]
    return cos_dup, sin_signed


def _build_program():
    import concourse.bass as bass
    from concourse import bacc, mybir
    import concourse.tile as tile

    f32 = mybir.dt.float32
    f32r = mybir.dt.float32r
    bf16 = mybir.dt.bfloat16
    ADD = mybir.AluOpType.add
    MULT = mybir.AluOpType.mult
    EXP = mybir.ActivationFunctionType.Exp
    SWAP16 = [(j + 16) % 32 for j in range(32)]
    NDK = D // 128  # contraction chunks for projections

    nc = bacc.Bacc("TRN2", target_bir_lowering=False, debug=False)
    # all inputs arrive pre-arranged partition-major so DMA descriptors are
    # large contiguous runs (small runs halve effective DMA bandwidth)
    xpm = nc.dram_tensor("xpm", [128, NQT, NDK, QT], bf16, kind="ExternalInput").ap()
    wq = nc.dram_tensor("wq", [128, NDK, DC], bf16, kind="ExternalInput").ap()
    wk = nc.dram_tensor("wk", [128, NDK, DC], bf16, kind="ExternalInput").ap()
    wv = nc.dram_tensor("wv", [128, NDK, DC], bf16, kind="ExternalInput").ap()
    wo = nc.dram_tensor("wo", [128, GQ, D], bf16, kind="ExternalInput").ap()
    csd = nc.dram_tensor("csd", [128, NQT, 2, QT], bf16, kind="ExternalInput").ap()
    tri = nc.dram_tensor("tri", [KC, KC], bf16, kind="ExternalInput").ap()
    ident = nc.dram_tensor("ident", [KC, KC], bf16, kind="ExternalInput").ap()
    outT = nc.dram_tensor("outT", [D, S], bf16, kind="ExternalOutput").ap()

    with tile.TileContext(nc) as tc:
        with tc.tile_pool(name="const", bufs=1) as const, \
             tc.tile_pool(name="tmp2", bufs=3) as tmp2, \
             tc.tile_pool(name="probs", bufs=8) as probs_pool, \
             tc.tile_pool(name="rsb", bufs=3) as rsb_pool, \
             tc.tile_pool(name="sums", bufs=2) as sums_pool, \
             tc.tile_pool(name="stage", bufs=4) as stage_pool, \
             tc.tile_pool(name="psb", bufs=6, space="PSUM") as psb, \
             tc.tile_pool(name="pso", bufs=2, space="PSUM") as pso:
            cs_sb = const.tile([128, NQT, 2, QT], bf16)
            tri_sb = const.tile([KC, KC], bf16)
            ident_sb = const.tile([KC, KC], bf16)
            wo_sb = const.tile([128, GQ, D], bf16)
            qT_sb = const.tile([128, GQ, S], bf16)
            kT_sb = const.tile([128, GQ, S], bf16)
            vaug_sb = const.tile([128, NKC, HPC * (HD + 1)], bf16)
            oT_sb = const.tile([128, GQ, S], bf16)
            xq_t = [const.tile([128, NDK, QT], bf16, name=f"x{qt}")
                    for qt in range(NQT)]
            wq_t = const.tile([128, NDK, DC], bf16, name="wqt")
            wk_t = const.tile([128, NDK, DC], bf16, name="wkt")
            wv_t = const.tile([128, NDK, DC], bf16, name="wvt")

            # Input DMA: priority-ordered slices round-robined over the three
            # DGE queues.  x is loaded by q-tile column chunk so the first
            # projection chain waits on ~2 MB, not the full 6.5 MB input set.
            nc.gpsimd.memset(vaug_sb[:, :, HD::(HD + 1)], 1.0)
            dma_items = []

            def xi(qt, k0, k1):
                return (xq_t[qt][:, k0:k1, :], xpm[:, qt, k0:k1, :])

            # qt0-critical order: the very first chain matmul gates on just
            # wq[kc 0-3] + x[kc 0-1] (one slice per queue)
            dma_items += [
                (wq_t[:, 0:4, :], wq[:, 0:4, :]), xi(0, 0, 2), xi(0, 2, 4),
                (wq_t[:, 4:8, :], wq[:, 4:8, :]), xi(0, 4, 6), xi(0, 6, 8),
                (wk_t[:, 0:4, :], wk[:, 0:4, :]),
                (wk_t[:, 4:8, :], wk[:, 4:8, :]),
                (cs_sb[:, 0, :, :], csd[:, 0, :, :]),
                (wv_t[:, 0:4, :], wv[:, 0:4, :]),
                (wv_t[:, 4:8, :], wv[:, 4:8, :]),
                (tri_sb, tri), (ident_sb, ident),
            ]
            for qt in range(1, NQT):
                dma_items.append(xi(qt, 0, 4))
                dma_items.append(xi(qt, 4, 8))
                dma_items.append((cs_sb[:, qt, :, :], csd[:, qt, :, :]))
                if qt == 1:
                    dma_items.append((wo_sb, wo))
            for i, (dst, src) in enumerate(dma_items):
                [nc.sync, nc.gpsimd, nc.scalar][i % 3].dma_start(dst, src)

            xT_sb = None  # x is addressed per q-tile chunk below
            wq_sb = [wq_t[:, kc, :] for kc in range(NDK)]
            wk_sb = [wk_t[:, kc, :] for kc in range(NDK)]
            wv_sb = [wv_t[:, kc, :] for kc in range(NDK)]

            dmaeng = [nc.sync, nc.gpsimd, nc.scalar]
            pending_op = []  # deferred out-projection work (one qt behind)

            def rope(ps, dst, q0):
                qt = q0 // QT
                shuf = tmp2.tile([128, QT], f32, tag="shuf")
                nc.vector.stream_shuffle(shuf, ps, SWAP16)
                m1 = tmp2.tile([128, QT], f32, tag="m1")
                nc.vector.tensor_tensor(m1, ps, cs_sb[:, qt, 0, :], MULT)
                m2 = tmp2.tile([128, QT], f32, tag="m2")
                nc.vector.tensor_tensor(m2, shuf, cs_sb[:, qt, 1, :], MULT)
                nc.vector.tensor_tensor(dst, m1, m2, ADD)

            def outproj_ec(qt, ec, last=False):
                q0 = qt * QT
                ps = psb.tile([128, QT], f32, tag="b", name="op")
                for g in range(GQ):
                    nc.tensor.matmul(
                        ps, (wo_sb[:, g, ec * 128:(ec + 1) * 128]),
                        (oT_sb[:, g, q0:q0 + QT]),
                        start=(g == 0), stop=(g == GQ - 1))
                ob = stage_pool.tile([128, QT], bf16, tag="ob")
                # ACT is exp-free by the final tile: split its drains DVE/ACT
                if last and ec % 2 == 1:
                    nc.scalar.copy(out=ob, in_=ps)
                else:
                    nc.vector.tensor_copy(out=ob, in_=ps)
                dmaeng[ec % 3].dma_start(
                    outT[ec * 128:(ec + 1) * 128, q0:q0 + QT], ob)

            def qk_chain(qt, g, which):
                q0 = qt * QT
                w_sb, dst = ((wq_sb, qT_sb) if which == "q" else (wk_sb, kT_sb))
                ps = psb.tile([128, QT], f32, tag="b", name=which)
                for kc in range(NDK):
                    st = dict(start=(kc == 0), stop=(kc == NDK - 1))
                    nc.tensor.matmul(
                        ps, (w_sb[kc][:, g * 128:(g + 1) * 128]),
                        (xq_t[qt][:, kc, :]), **st)
                rope(ps, dst[:, g, q0:q0 + QT], q0)

            def v_chain(qt, rp):
                ps_v = psb.tile([128, 2 * DC], f32, tag="b", name="v")
                for half in range(2):
                    j = 2 * rp + half
                    for kc in range(NDK):
                        st = dict(start=(kc == 0), stop=(kc == NDK - 1))
                        nc.tensor.matmul(
                            ps_v[:, half * DC:(half + 1) * DC],
                            (xq_t[qt][:, kc, j * 128:(j + 1) * 128]),
                            (wv_sb[kc]), **st)
                for half in range(2):
                    rc = 4 * qt + 2 * rp + half
                    for h in range(HPC):
                        nc.vector.tensor_copy(
                            out=vaug_sb[:, rc, h * (HD + 1):h * (HD + 1) + HD],
                            in_=ps_v[:, half * DC + h * HD:half * DC + (h + 1) * HD])

            def proj_units(qt):
                units = []
                for g in range(GQ):
                    units.append(lambda g=g: qk_chain(qt, g, "k"))
                    units.append(lambda g=g: qk_chain(qt, g, "q"))
                for rp in range(2):
                    units.append(lambda rp=rp: v_chain(qt, rp))
                return units

            for u in proj_units(0):
                u()
            pending = []  # heavy fill units interleaved into attention
            for qt in range(NQT):
                q0 = qt * QT
                nkc = (q0 + QT) // KC
                if qt + 1 < NQT:
                    pending += proj_units(qt + 1)
                if qt > 0:
                    pending += [lambda ec=ec, pq=qt - 1: outproj_ec(pq, ec)
                                for ec in range(D // 128)]
                # interleave heavy units across this tile's attention chunks
                n_units = GQ * nkc
                quota, acc = len(pending) / n_units, 0.0

                # ---- causal flash attention for this q-tile ----
                for g in range(GQ):
                    ps_o = [pso.tile([HD + 1, QT], f32, tag="o",
                                     name=f"o{g}{a}") for a in range(2)]
                    pend = []  # (probs pair, kc, qlo) awaiting AV matmul

                    def emit_av(item, last, g=g, ps_o=ps_o):
                        pr, kc_, qlo_ = item
                        for a in range(2):
                            h = 2 * g + a
                            nc.tensor.matmul(
                                ps_o[a][:, qlo_:QT],
                                (vaug_sb[:, kc_, h * (HD + 1):(h + 1) * (HD + 1)]),
                                (pr[a][:, qlo_:QT]),
                                start=(kc_ == 0), stop=last)

                    for kc in range(nkc):
                        k0 = kc * KC
                        qlo = max(0, k0 - q0)
                        masked = k0 >= q0
                        ps_s = [psb.tile([128, QT], f32, tag="b",
                                         name=f"s{a}") for a in range(2)]
                        for a in range(2):
                            nc.tensor.matmul(
                                ps_s[a][:, qlo:QT],
                                (kT_sb[a * HD:(a + 1) * HD, g, k0:k0 + KC]),
                                (qT_sb[a * HD:(a + 1) * HD, g, q0 + qlo:q0 + QT]),
                                start=True, stop=not masked,
                                skip_group_check=masked)
                            if masked:
                                # diagonal-band mask folded into the PSUM
                                # accumulation: ps_s[:, qlo:qlo+KC] += I.T @ tri
                                nc.tensor.matmul(
                                    ps_s[a][:, qlo:qlo + KC], ident_sb, tri_sb,
                                    start=False, stop=True,
                                    skip_group_check=True)
                        pr = [probs_pool.tile([128, QT], bf16, tag="p",
                                              name=f"p{a}") for a in range(2)]
                        for a in range(2):
                            nc.scalar.activation(
                                pr[a][:, qlo:QT], ps_s[a][:, qlo:QT], EXP)
                        pend.append((pr, kc, qlo))
                        if len(pend) > 2:
                            emit_av(pend.pop(0), last=False)
                        acc += quota
                        while pending and acc >= 1.0:
                            pending.pop(0)()
                            acc -= 1.0
                    while pend:
                        emit_av(pend.pop(0), last=(len(pend) == 0))

                    # ---- on-chip normalization for this (qt, g) ----
                    sums_t = sums_pool.tile([1, 2, QT], f32, tag="sm")
                    for a in range(2):
                        nc.vector.tensor_copy(
                            out=sums_t[0:1, a, :], in_=ps_o[a][HD:HD + 1, :])
                    recip_t = rsb_pool.tile([1, 2, QT], f32, tag="rr")
                    nc.vector.reciprocal_approx_fast(
                        out=recip_t[0:1, :, :], in_=sums_t[0:1, :, :])
                    for a in range(2):
                        r_a = rsb_pool.tile([HD, QT], f32, tag="r")
                        nc.gpsimd.partition_broadcast(r_a, recip_t[0:1, a, :])
                        nc.vector.tensor_tensor(
                            oT_sb[a * HD:(a + 1) * HD, g, q0:q0 + QT],
                            ps_o[a][0:HD, :], r_a, MULT)
                while pending:
                    pending.pop(0)()
            for ec in range(D // 128):
                outproj_ec(NQT - 1, ec, last=True)

    nc.finalize()
    return nc


def kernel(x, wq, wk, wv, wo):
    from concourse import bass_utils

    if os.environ.get("BASS_TRACE"):
        _install_axon_ntff_hook()

    x = np.asarray(x, dtype=np.float32)
    wq = np.asarray(wq, dtype=np.float32)
    wk = np.asarray(wk, dtype=np.float32)
    wv = np.asarray(wv, dtype=np.float32)
    wo = np.asarray(wo, dtype=np.float32)

    # Host prep: weight slicing + rope column permutation + tables, all
    # pre-arranged partition-major ([128, ...free]) for large DMA runs.
    perm_l = _rope_perm_local()
    perm = np.concatenate([h * HD + perm_l for h in range(NH)])  # [D]
    scale = 1.0 / np.sqrt(HD)
    wq_p = (wq[:, perm] * scale).astype(BF16)
    wk_p = wk[:, perm].astype(BF16)
    wv_b = wv.astype(BF16)
    wo_b = wo.astype(BF16)
    cos_dup, sin_signed = _rope_tables()
    kl = np.arange(KC)[:, None]
    ql = np.arange(KC)[None, :]
    tri = np.where(ql >= kl, 0.0, MASKVAL).astype(BF16)
    ident = np.eye(KC, dtype=np.float32).astype(BF16)
    csd = np.ascontiguousarray(
        np.stack([cos_dup.reshape(128, NQT, QT), sin_signed.reshape(128, NQT, QT)],
                 axis=2)).astype(BF16)  # [128, NQT, 2, QT]

    def pmaj(w):  # [NDK*128, C] -> [128, NDK, C]
        return np.ascontiguousarray(
            w.reshape(NDK, 128, w.shape[1]).transpose(1, 0, 2))

    # x[b] [S, D] -> xpm [128, NQT, NDK, QT]: xpm[p,qt,kc,n] = x[b][qt*QT+n, kc*128+p]
    xpms = [np.ascontiguousarray(
        x[b].reshape(NQT, QT, NDK, 128).transpose(3, 0, 2, 1)).astype(BF16)
        for b in range(B)]

    in_maps = []
    for i in range(NCORES):
        b, g = divmod(i, HPC)
        cs = slice(g * DC, (g + 1) * DC)
        wo_g = wo_b[cs, :]  # [DC, D] -> [128, GQ, D]
        wo_pm = np.ascontiguousarray(
            wo_g.reshape(GQ, 128, D).transpose(1, 0, 2))
        in_maps.append({
            "xpm": xpms[b],
            "wq": pmaj(np.ascontiguousarray(wq_p[:, cs])),
            "wk": pmaj(np.ascontiguousarray(wk_p[:, cs])),
            "wv": pmaj(np.ascontiguousarray(wv_b[:, cs])),
            "wo": wo_pm,
            "csd": csd,
            "tri": tri,
            "ident": ident,
        })

    if "nc" not in _CACHE:
        _CACHE["nc"] = _build_program()
    nc = _CACHE["nc"]

    res = bass_utils.run_bass_kernel_spmd(nc, in_maps, core_ids=list(range(NCORES)))
    _CACHE["last_exec_time_ns"] = res.exec_time_ns
    _CACHE["last_res"] = res

    out = np.empty((B, S, D), dtype=np.float32)
    for b in range(B):
        acc = np.asarray(res.results[b * HPC]["outT"], dtype=np.float32)
        for g in range(1, HPC):
            acc += np.asarray(res.results[b * HPC + g]["outT"], dtype=np.float32)
        out[b] = acc.T
    return out


# revision 47
# speedup vs baseline: 1.0250x; 1.0250x over previous
"""Trainium2 Bass kernel for causal multi-head attention with interleaved RoPE.

Problem: B=2, S=2048, D=1024, 16 heads x 64 dims, causal, rope theta=1e4.

Sharding (8 cores): 2-way batch x 4-way head tensor-parallel.
  core i: batch b = i // 4, head group g = i % 4 (heads 4g..4g+3, dims 256).
  Each core computes q/k/v for its heads from x[b], runs causal flash
  attention, and produces a partial output projection outT = wo_g.T-slice
  contribution [D, S].  Host sums the 4 partials per batch and transposes.

v2 layout notes:
  - All matmul operands are bf16 (PSUM accumulation stays f32); tolerance is
    2e-2 so bf16's ~0.2% error is far inside budget.  bf16 streams at 1
    cycle/row on the PE for any N and halves SBUF/DMA traffic.
  - qT/kT are computed transposed ([dim, row]) via lhsT=weights, rhs=x^T.
  - RoPE pairs are de-interleaved on the host by permuting weight columns so
    pair partners sit 16 partitions apart (within a 32-partition quadrant),
    making the partner fetch a single DVE stream_shuffle.
  - Scores are computed transposed (S^T[k, q]) so the AV matmul needs no
    transposes; the softmax normalizer comes from a ones-row appended to V.
  - Causality: only k-chunks up to the diagonal are processed; diagonal-band
    128x128 blocks get a -60 triangular mask add before exp.
  - Attention is software-pipelined: the AV matmul for chunk kc is emitted
    after the score matmuls for chunk kc+1, so the Tensor engine never waits
    on the Scalar-engine exp.
  - Normalization is per-(qt,g) and stays on-chip: sums row -> K=1 broadcast
    matmul -> reciprocal_approx_fast -> fused multiply into bf16 oT.
"""

import os
import sys

sys.path.insert(0, "/opt/trn_rl_repo")

import numpy as np
import ml_dtypes

B = 2
S = 2048
D = 1024
NH = 16
HD = 64
THETA = 10000.0
NCORES = 8
HPC = 4  # heads per core
DC = HPC * HD  # 256 dims per core
GQ = 2  # 128-partition groups per core for q/k/o dims (DC/128)
QT = 512  # query tile (free dim)
NQT = S // QT
KC = 128  # key chunk (partition dim)
NKC = S // KC
NDK = D // 128  # contraction chunks for projections
MASKVAL = -60.0
BF16 = ml_dtypes.bfloat16

_CACHE = {}


def _install_axon_ntff_hook():
    """Register antenv.axon_hooks so trace=True (BASS_TRACE=1) works."""
    import types

    if "antenv.axon_hooks" in sys.modules:
        return
    m = types.ModuleType("antenv.axon_hooks")
    _hook = [None]
    m.set_axon_ntff_profile_hook = lambda h: _hook.__setitem__(0, h)
    m.get_axon_ntff_profile_hook = lambda: _hook[0]
    sys.modules["antenv.axon_hooks"] = m
    try:
        import antenv

        antenv.axon_hooks = m
        from trn_agent_boot.trn_boot import _ntff_profile_via_ctypes

        hook = _ntff_profile_via_ctypes("/opt/axon/libaxon_pjrt.so")
        if hook is not None:
            m.set_axon_ntff_profile_hook(hook)
    except Exception:
        pass


def _rope_perm_local():
    """Permutation of one head's 64 dims: original interleaved pair (2i, 2i+1)
    -> t0 at quadrant*32 + (i%16), t1 at quadrant*32 + 16 + (i%16), with
    quadrant = i // 16.  Returns perm such that new[j] = old[perm[j]]."""
    perm = np.zeros(HD, dtype=np.int64)
    for i in range(HD // 2):
        qd, r = divmod(i, 16)
        perm[qd * 32 + r] = 2 * i
        perm[qd * 32 + 16 + r] = 2 * i + 1
    return perm


def _rope_tables():
    """cos_dup/sin_signed [128, S]: per-partition rope tables matching the
    de-interleaved layout (pattern repeats every 64 partitions)."""
    inv_freq = 1.0 / (THETA ** (np.arange(0, HD, 2, dtype=np.float64) / HD))  # [32]
    pos = np.arange(S, dtype=np.float64)
    ang = pos[None, :] * inv_freq[:, None]  # [32, S]
    cos = np.cos(ang)
    sin = np.sin(ang)
    cos_dup = np.zeros((128, S), dtype=np.float32)
    sin_signed = np.zeros((128, S), dtype=np.float32)
    for p in range(128):
        d = p % HD
        qd, r0 = divmod(d, 32)
        if r0 < 16:
            i = qd * 16 + r0
            cos_dup[p] = cos[i]
            sin_signed[p] = -sin[i]
        else:
            i = qd * 16 + (r0 - 16)
            cos_dup[p] = cos[i]
            sin_signed[p] = sin[i]
    return cos_dup, sin_signed


def _build_program():
    import concourse.bass as bass
    from concourse import bacc, mybir
    import concourse.tile as tile

    f32 = mybir.dt.float32
    f32r = mybir.dt.float32r
    bf16 = mybir.dt.bfloat16
    ADD = mybir.AluOpType.add
    MULT = mybir.AluOpType.mult
    EXP = mybir.ActivationFunctionType.Exp
    SWAP16 = [(j + 16) % 32 for j in range(32)]
    NDK = D // 128  # contraction chunks for projections

    nc = bacc.Bacc("TRN2", target_bir_lowering=False, debug=False)
    # all inputs arrive pre-arranged partition-major so DMA descriptors are
    # large contiguous runs (small runs halve effective DMA bandwidth)
    xpm = nc.dram_tensor("xpm", [128, NQT, NDK, QT], bf16, kind="ExternalInput").ap()
    wq = nc.dram_tensor("wq", [128, NDK, DC], bf16, kind="ExternalInput").ap()
    wk = nc.dram_tensor("wk", [128, NDK, DC], bf16, kind="ExternalInput").ap()
    wv = nc.dram_tensor("wv", [128, NDK, DC], bf16, kind="ExternalInput").ap()
    wo = nc.dram_tensor("wo", [128, GQ, D], bf16, kind="ExternalInput").ap()
    csd = nc.dram_tensor("csd", [128, NQT, 2, QT], bf16, kind="ExternalInput").ap()
    tri = nc.dram_tensor("tri", [KC, KC], bf16, kind="ExternalInput").ap()
    ident = nc.dram_tensor("ident", [KC, KC], bf16, kind="ExternalInput").ap()
    outT = nc.dram_tensor("outT", [D, S], bf16, kind="ExternalOutput").ap()

    with tile.TileContext(nc) as tc:
        with tc.tile_pool(name="const", bufs=1) as const, \
             tc.tile_pool(name="tmp2", bufs=3) as tmp2, \
             tc.tile_pool(name="probs", bufs=8) as probs_pool, \
             tc.tile_pool(name="rsb", bufs=3) as rsb_pool, \
             tc.tile_pool(name="sums", bufs=2) as sums_pool, \
             tc.tile_pool(name="stage", bufs=4) as stage_pool, \
             tc.tile_pool(name="psb", bufs=5, space="PSUM") as psb, \
             tc.tile_pool(name="pso", bufs=3, space="PSUM") as pso:
            cs_sb = const.tile([128, NQT, 2, QT], bf16)
            tri_sb = const.tile([KC, KC], bf16)
            ident_sb = const.tile([KC, KC], bf16)
            wo_sb = const.tile([128, GQ, D], bf16)
            qT_sb = const.tile([128, GQ, S], bf16)
            kT_sb = const.tile([128, GQ, S], bf16)
            vaug_sb = const.tile([128, NKC, HPC * (HD + 1)], bf16)
            oT_sb = const.tile([128, GQ, S], bf16)
            xq_t = [const.tile([128, NDK, QT], bf16, name=f"x{qt}")
                    for qt in range(NQT)]
            wq_t = const.tile([128, NDK, DC], bf16, name="wqt")
            wk_t = const.tile([128, NDK, DC], bf16, name="wkt")
            wv_t = const.tile([128, NDK, DC], bf16, name="wvt")

            # Input DMA: priority-ordered slices round-robined over the three
            # DGE queues.  x is loaded by q-tile column chunk so the first
            # projection chain waits on ~2 MB, not the full 6.5 MB input set.
            nc.gpsimd.memset(vaug_sb[:, :, HD::(HD + 1)], 1.0)
            dma_items = []

            def xi(qt, k0, k1):
                return (xq_t[qt][:, k0:k1, :], xpm[:, qt, k0:k1, :])

            # qt0-critical order: the very first chain matmul gates on just
            # wq[kc 0-3] + x[kc 0-1] (one slice per queue)
            dma_items += [
                (wq_t[:, 0:4, :], wq[:, 0:4, :]), xi(0, 0, 2), xi(0, 2, 4),
                (wq_t[:, 4:8, :], wq[:, 4:8, :]), xi(0, 4, 6), xi(0, 6, 8),
                (wk_t[:, 0:4, :], wk[:, 0:4, :]),
                (wk_t[:, 4:8, :], wk[:, 4:8, :]),
                (cs_sb[:, 0, :, :], csd[:, 0, :, :]),
                (wv_t[:, 0:4, :], wv[:, 0:4, :]),
                (wv_t[:, 4:8, :], wv[:, 4:8, :]),
                (tri_sb, tri), (ident_sb, ident),
            ]
            for qt in range(1, NQT):
                dma_items.append(xi(qt, 0, 4))
                dma_items.append(xi(qt, 4, 8))
                dma_items.append((cs_sb[:, qt, :, :], csd[:, qt, :, :]))
                if qt == 1:
                    dma_items.append((wo_sb, wo))
            for i, (dst, src) in enumerate(dma_items):
                [nc.sync, nc.gpsimd, nc.scalar][i % 3].dma_start(dst, src)

            xT_sb = None  # x is addressed per q-tile chunk below
            wq_sb = [wq_t[:, kc, :] for kc in range(NDK)]
            wk_sb = [wk_t[:, kc, :] for kc in range(NDK)]
            wv_sb = [wv_t[:, kc, :] for kc in range(NDK)]

            dmaeng = [nc.sync, nc.gpsimd, nc.scalar]
            pending_op = []  # deferred out-projection work (one qt behind)

            def rope(ps, dst, q0):
                qt = q0 // QT
                shuf = tmp2.tile([128, QT], f32, tag="shuf")
                nc.vector.stream_shuffle(shuf, ps, SWAP16)
                m1 = tmp2.tile([128, QT], f32, tag="m1")
                nc.vector.tensor_tensor(m1, ps, cs_sb[:, qt, 0, :], MULT)
                m2 = tmp2.tile([128, QT], f32, tag="m2")
                nc.vector.tensor_tensor(m2, shuf, cs_sb[:, qt, 1, :], MULT)
                nc.vector.tensor_tensor(dst, m1, m2, ADD)

            def outproj_ec(qt, ec, last=False):
                q0 = qt * QT
                ps = psb.tile([128, QT], f32, tag="b", name="op")
                for g in range(GQ):
                    nc.tensor.matmul(
                        ps, (wo_sb[:, g, ec * 128:(ec + 1) * 128]),
                        (oT_sb[:, g, q0:q0 + QT]),
                        start=(g == 0), stop=(g == GQ - 1))
                ob = stage_pool.tile([128, QT], bf16, tag="ob")
                # ACT is exp-free by the final tile: split its drains DVE/ACT
                if last and ec % 2 == 1:
                    nc.scalar.copy(out=ob, in_=ps)
                else:
                    nc.vector.tensor_copy(out=ob, in_=ps)
                dmaeng[ec % 3].dma_start(
                    outT[ec * 128:(ec + 1) * 128, q0:q0 + QT], ob)

            def qk_chain(qt, g, which):
                q0 = qt * QT
                w_sb, dst = ((wq_sb, qT_sb) if which == "q" else (wk_sb, kT_sb))
                ps = psb.tile([128, QT], f32, tag="b", name=which)
                for kc in range(NDK):
                    st = dict(start=(kc == 0), stop=(kc == NDK - 1))
                    nc.tensor.matmul(
                        ps, (w_sb[kc][:, g * 128:(g + 1) * 128]),
                        (xq_t[qt][:, kc, :]), **st)
                rope(ps, dst[:, g, q0:q0 + QT], q0)

            def v_chain(qt, rp):
                ps_v = psb.tile([128, 2 * DC], f32, tag="b", name="v")
                for half in range(2):
                    j = 2 * rp + half
                    for kc in range(NDK):
                        st = dict(start=(kc == 0), stop=(kc == NDK - 1))
                        nc.tensor.matmul(
                            ps_v[:, half * DC:(half + 1) * DC],
                            (xq_t[qt][:, kc, j * 128:(j + 1) * 128]),
                            (wv_sb[kc]), **st)
                for half in range(2):
                    rc = 4 * qt + 2 * rp + half
                    for h in range(HPC):
                        nc.vector.tensor_copy(
                            out=vaug_sb[:, rc, h * (HD + 1):h * (HD + 1) + HD],
                            in_=ps_v[:, half * DC + h * HD:half * DC + (h + 1) * HD])

            def proj_units(qt):
                units = []
                for g in range(GQ):
                    units.append(lambda g=g: qk_chain(qt, g, "k"))
                    units.append(lambda g=g: qk_chain(qt, g, "q"))
                for rp in range(2):
                    units.append(lambda rp=rp: v_chain(qt, rp))
                return units

            for u in proj_units(0):
                u()
            pending = []  # heavy fill units interleaved into attention
            for qt in range(NQT):
                q0 = qt * QT
                nkc = (q0 + QT) // KC
                if qt + 1 < NQT:
                    pending += proj_units(qt + 1)
                if qt > 0:
                    pending += [lambda ec=ec, pq=qt - 1: outproj_ec(pq, ec)
                                for ec in range(D // 128)]
                # interleave heavy units across this tile's attention chunks
                n_units = GQ * nkc
                quota, acc = len(pending) / n_units, 0.0

                # ---- causal flash attention for this q-tile ----
                for g in range(GQ):
                    ps_o = [pso.tile([HD + 1, QT], f32, tag="o",
                                     name=f"o{g}{a}") for a in range(2)]
                    pend = []  # (probs pair, kc, qlo) awaiting AV matmul

                    def emit_av(item, last, g=g, ps_o=ps_o):
                        pr, kc_, qlo_ = item
                        for a in range(2):
                            h = 2 * g + a
                            nc.tensor.matmul(
                                ps_o[a][:, qlo_:QT],
                                (vaug_sb[:, kc_, h * (HD + 1):(h + 1) * (HD + 1)]),
                                (pr[a][:, qlo_:QT]),
                                start=(kc_ == 0), stop=last)

                    for kc in range(nkc):
                        k0 = kc * KC
                        qlo = max(0, k0 - q0)
                        masked = k0 >= q0
                        ps_s = [psb.tile([128, QT], f32, tag="b",
                                         name=f"s{a}") for a in range(2)]
                        for a in range(2):
                            nc.tensor.matmul(
                                ps_s[a][:, qlo:QT],
                                (kT_sb[a * HD:(a + 1) * HD, g, k0:k0 + KC]),
                                (qT_sb[a * HD:(a + 1) * HD, g, q0 + qlo:q0 + QT]),
                                start=True, stop=not masked,
                                skip_group_check=masked)
                            if masked:
                                # diagonal-band mask folded into the PSUM
                                # accumulation: ps_s[:, qlo:qlo+KC] += I.T @ tri
                                nc.tensor.matmul(
                                    ps_s[a][:, qlo:qlo + KC], ident_sb, tri_sb,
                                    start=False, stop=True,
                                    skip_group_check=True)
                        pr = [probs_pool.tile([128, QT], bf16, tag="p",
                                              name=f"p{a}") for a in range(2)]
                        for a in range(2):
                            nc.scalar.activation(
                                pr[a][:, qlo:QT], ps_s[a][:, qlo:QT], EXP)
                        pend.append((pr, kc, qlo))
                        if len(pend) > 2:
                            emit_av(pend.pop(0), last=False)
                        acc += quota
                        while pending and acc >= 1.0:
                            pending.pop(0)()
                            acc -= 1.0
                    while pend:
                        emit_av(pend.pop(0), last=(len(pend) == 0))

                    # ---- on-chip normalization for this (qt, g) ----
                    sums_t = sums_pool.tile([1, 2, QT], f32, tag="sm")
                    for a in range(2):
                        nc.vector.tensor_copy(
                            out=sums_t[0:1, a, :], in_=ps_o[a][HD:HD + 1, :])
                    recip_t = rsb_pool.tile([1, 2, QT], f32, tag="rr")
                    nc.vector.reciprocal_approx_fast(
                        out=recip_t[0:1, :, :], in_=sums_t[0:1, :, :])
                    for a in range(2):
                        r_a = rsb_pool.tile([HD, QT], f32, tag="r")
                        nc.gpsimd.partition_broadcast(r_a, recip_t[0:1, a, :])
                        nc.vector.tensor_tensor(
                            oT_sb[a * HD:(a + 1) * HD, g, q0:q0 + QT],
                            ps_o[a][0:HD, :], r_a, MULT)
                while pending:
                    pending.pop(0)()
            for ec in range(D // 128):
                outproj_ec(NQT - 1, ec, last=True)

    nc.finalize()
    return nc


def kernel(x, wq, wk, wv, wo):
    from concourse import bass_utils

    if os.environ.get("BASS_TRACE"):
        _install_axon_ntff_hook()

    x = np.asarray(x, dtype=np.float32)
    wq = np.asarray(wq, dtype=np.float32)
    wk = np.asarray(wk, dtype=np.float32)
    wv = np.asarray(wv, dtype=np.float32)
    wo = np.asarray(wo, dtype=np.float32)

    # Host prep: weight slicing + rope column permutation + tables, all
    # pre-arranged partition-major ([128, ...free]) for large DMA runs.
    perm_l = _rope_perm_local()
    perm = np.concatenate([h * HD + perm_l for h in range(NH)])  # [D]
    scale = 1.0 / np.sqrt(HD)
    wq_p = (wq[:, perm] * scale).astype(BF16)
    wk_p = wk[:, perm].astype(BF16)
    wv_b = wv.astype(BF16)
    wo_b = wo.astype(BF16)
    cos_dup, sin_signed = _rope_tables()
    kl = np.arange(KC)[:, None]
    ql = np.arange(KC)[None, :]
    tri = np.where(ql >= kl, 0.0, MASKVAL).astype(BF16)
    ident = np.eye(KC, dtype=np.float32).astype(BF16)
    csd = np.ascontiguousarray(
        np.stack([cos_dup.reshape(128, NQT, QT), sin_signed.reshape(128, NQT, QT)],
                 axis=2)).astype(BF16)  # [128, NQT, 2, QT]

    def pmaj(w):  # [NDK*128, C] -> [128, NDK, C]
        return np.ascontiguousarray(
            w.reshape(NDK, 128, w.shape[1]).transpose(1, 0, 2))

    # x[b] [S, D] -> xpm [128, NQT, NDK, QT]: xpm[p,qt,kc,n] = x[b][qt*QT+n, kc*128+p]
    xpms = [np.ascontiguousarray(
        x[b].reshape(NQT, QT, NDK, 128).transpose(3, 0, 2, 1)).astype(BF16)
        for b in range(B)]

    in_maps = []
    for i in range(NCORES):
        b, g = divmod(i, HPC)
        cs = slice(g * DC, (g + 1) * DC)
        wo_g = wo_b[cs, :]  # [DC, D] -> [128, GQ, D]
        wo_pm = np.ascontiguousarray(
            wo_g.reshape(GQ, 128, D).transpose(1, 0, 2))
        in_maps.append({
            "xpm": xpms[b],
            "wq": pmaj(np.ascontiguousarray(wq_p[:, cs])),
            "wk": pmaj(np.ascontiguousarray(wk_p[:, cs])),
            "wv": pmaj(np.ascontiguousarray(wv_b[:, cs])),
            "wo": wo_pm,
            "csd": csd,
            "tri": tri,
            "ident": ident,
        })

    if "nc" not in _CACHE:
        _CACHE["nc"] = _build_program()
    nc = _CACHE["nc"]

    res = bass_utils.run_bass_kernel_spmd(nc, in_maps, core_ids=list(range(NCORES)))
    _CACHE["last_exec_time_ns"] = res.exec_time_ns
    _CACHE["last_res"] = res

    out = np.empty((B, S, D), dtype=np.float32)
    for b in range(B):
        acc = np.asarray(res.results[b * HPC]["outT"], dtype=np.float32)
        for g in range(1, HPC):
            acc += np.asarray(res.results[b * HPC + g]["outT"], dtype=np.float32)
        out[b] = acc.T
    return out
